# revision 1
# baseline (speedup 1.0000x reference)
"""HGT layer (2 node types, 2 relations) on 8 Trainium2 cores.

Strategy (dst-sharded, fully fused single pass):
  - Each core owns 12500 destination nodes of each type (out_a rows and
    out_b rows). Edges are partitioned by destination shard on the host
    and sorted into groups of 128 consecutive destination nodes, padded
    to a fixed per-group edge capacity C = T*128.
  - Per group, the kernel gathers source-node x rows (indirect DMA),
    projects K/V on the fly (PE), expands Q to edges via a one-hot
    (edge x dst) matrix (PE matmul), computes per-edge attention logits,
    exponentiates (no max-subtraction needed: logits are O(3)), and
    performs segment-sum (denominator) and weighted-V aggregation as
    PE matmuls against the one-hot matrix.  Normalization, the relation
    mixing matmul (Wmsg), skip connection, relu and layernorm are all
    fused in the same group iteration; nothing round-trips to DRAM.
  - The per-head attention scale (SCALE * sigmoid(mu_h)) is folded into
    Wq on the host.
"""

import numpy as np

import concourse.bacc as bacc
import concourse.bass as bass
import concourse.mybir as mybir
import concourse.tile as tile
from concourse.bass import ds
from concourse.masks import make_identity

N = 100000
D = 256
H = 8
DH = 32
M = 8            # cores
NSH = N // M     # 12500 dst rows per core per type
G = 98           # dst groups of 128 per core (98*128 = 12544)
NPAD = G * 128   # 12544
EPS = 1e-5
F32 = mybir.dt.float32
I32 = mybir.dt.int32
AF = mybir.ActivationFunctionType
OP = mybir.AluOpType


# ----------------------------------------------------------------- host prep

def _pack_edges(src, dst, T):
    """Partition edges by dst shard, group by 128 consecutive dsts, pad to
    T*128 slots per group.  Returns src_idx [M, NPAD, T] int32 and
    dstl [M, NPAD, T] float32 (dst-local-in-group; 999.0 for padding).
    Slot s of group g maps to SBUF (partition p = s % 128, column t = s // 128),
    i.e. row g*128 + p, col t of the packed array."""
    order = np.argsort(dst, kind="stable")
    s_sorted = src[order].astype(np.int64)
    d_sorted = dst[order].astype(np.int64)

    core = d_sorted // NSH
    local = d_sorted - core * NSH
    grp = local // 128
    dloc = local - grp * 128
    key = core * G + grp
    # rank of each edge within its (core, group)
    first = np.r_[0, np.flatnonzero(np.diff(key)) + 1]
    starts = np.zeros(len(key), dtype=np.int64)
    starts[first] = first
    starts = np.maximum.accumulate(starts)
    slot = np.arange(len(key), dtype=np.int64) - starts

    maxslot = int(slot.max()) if len(slot) else 0
    assert maxslot < T * 128, f"edge capacity exceeded: {maxslot + 1} > {T * 128}"

    src_arr = np.zeros((M * G, T * 128), dtype=np.int32)
    dst_arr = np.full((M * G, T * 128), 999.0, dtype=np.float32)
    src_arr[key, slot] = s_sorted
    dst_arr[key, slot] = dloc
    # [MG, T, 128] -> [MG, 128, T] -> [M, NPAD, T]
    src_arr = src_arr.reshape(M * G, T, 128).transpose(0, 2, 1)
    dst_arr = dst_arr.reshape(M * G, T, 128).transpose(0, 2, 1)
    return (src_arr.reshape(M, NPAD, T).copy(),
            dst_arr.reshape(M, NPAD, T).copy())


def _edge_capacity(dst):
    d = np.sort(dst.astype(np.int64))
    core = d // NSH
    grp = (d - core * NSH) // 128
    key = core * G + grp
    _, counts = np.unique(key, return_counts=True)
    return int(counts.max())


def _shard_rows(x):
    """[N, D] -> [M, NPAD, D], zero padded."""
    out = np.zeros((M, NPAD, D), dtype=x.dtype)
    for m in range(M):
        out[m, :NSH] = x[m * NSH:(m + 1) * NSH]
    return out


# ------------------------------------------------------------- bass program

def build_program(T, nfull=N, npad=NPAD):
    nc = bacc.Bacc("TRN2", target_bir_lowering=False, debug=False)
    g_iters = npad // 128

    def drt(name, shape, dtype=F32, kind="ExternalInput"):
        return nc.dram_tensor(name, shape, dtype, kind=kind)

    xa_full = drt("xa_full", [nfull, D])
    xb_full = drt("xb_full", [nfull, D])
    xa_dst = drt("xa_dst", [npad, D])
    xb_dst = drt("xb_dst", [npad, D])
    iota_row = drt("iota_row", [128, 128])

    rels = []
    for r in ("ab", "ba"):
        rels.append(dict(
            name=r,
            src=drt(f"src_{r}", [npad, T], I32),
            dstl=drt(f"dstl_{r}", [npad, T]),
            wq=drt(f"wq_{r}", [D, D]),
            wk=drt(f"wk_{r}", [D, D]),
            wv=drt(f"wv_{r}", [D, D]),
            wmsg=drt(f"wmsg_{r}", [D, D]),
            wskip=drt(f"wskip_{r}", [D, D]),
            bskip=drt(f"bskip_{r}", [1, D]),
            gln=drt(f"gln_{r}", [128, D]),
            bln=drt(f"bln_{r}", [128, D]),
            out=drt(f"out_{r}", [npad, D], kind="ExternalOutput"),
        ))
    rels[0]["xfull"] = xa_full   # ab: src type a
    rels[0]["xdst"] = xb_dst     # ab: dst type b
    rels[1]["xfull"] = xb_full
    rels[1]["xdst"] = xa_dst

    with tile.TileContext(nc) as tc:
        with (
            tc.tile_pool(name="const", bufs=1) as cp,
            tc.tile_pool(name="sbuf", bufs=2) as sp,
            tc.tile_pool(name="psum", bufs=1, space="PSUM") as pp,
            tc.tile_pool(name="psum3", bufs=2, space="PSUM") as pp3,
        ):
            ident = cp.tile([128, 128], F32)
            make_identity(nc, ident[:])
            iota = cp.tile([128, 128], F32)
            nc.sync.dma_start(out=iota[:], in_=iota_row[:])
            ones1 = cp.tile([1, 128], F32)
            nc.gpsimd.memset(ones1[:], 1.0)

            for rel in rels:
                # --- static per-relation weights
                wq = cp.tile([128, 2, D], F32, tag="wq")
                wk = cp.tile([128, 2, D], F32, tag="wk")
                wv = cp.tile([128, 2, D], F32, tag="wv")
                wmsg = cp.tile([128, 2, D], F32, tag="wmsg")
                wskip = cp.tile([128, 2, D], F32, tag="wskip")
                for c in range(2):
                    nc.sync.dma_start(out=wq[:, c, :], in_=rel["wq"][c * 128:(c + 1) * 128, :])
                    nc.sync.dma_start(out=wk[:, c, :], in_=rel["wk"][c * 128:(c + 1) * 128, :])
                    nc.sync.dma_start(out=wv[:, c, :], in_=rel["wv"][c * 128:(c + 1) * 128, :])
                    nc.sync.dma_start(out=wmsg[:, c, :], in_=rel["wmsg"][c * 128:(c + 1) * 128, :])
                    nc.sync.dma_start(out=wskip[:, c, :], in_=rel["wskip"][c * 128:(c + 1) * 128, :])
                bskip = cp.tile([1, D], F32, tag="bskip")
                nc.sync.dma_start(out=bskip[:], in_=rel["bskip"][:])
                gln = cp.tile([128, D], F32, tag="gln")
                bln = cp.tile([128, D], F32, tag="bln")
                nc.sync.dma_start(out=gln[:], in_=rel["gln"][:])
                nc.sync.dma_start(out=bln[:], in_=rel["bln"][:])

                xfull, xdst, srcd, dstd, outd = (
                    rel["xfull"], rel["xdst"], rel["src"], rel["dstl"], rel["out"])

                with tc.For_i(0, npad, 128) as g:
                    # ---- loads
                    xd = sp.tile([128, D], F32, tag="xd")
                    nc.sync.dma_start(out=xd[:], in_=xdst[ds(g, 128), :])
                    sidx = sp.tile([128, T], I32, tag="sidx")
                    nc.sync.dma_start(out=sidx[:], in_=srcd[ds(g, 128), :])
                    dcol = sp.tile([128, T], F32, tag="dcol")
                    nc.sync.dma_start(out=dcol[:], in_=dstd[ds(g, 128), :])
                    xg = sp.tile([128, T, D], F32, tag="xg")
                    for t in range(T):
                        nc.gpsimd.indirect_dma_start(
                            out=xg[:, t, :], out_offset=None,
                            in_=xfull[:],
                            in_offset=bass.IndirectOffsetOnAxis(
                                ap=sidx[:, t:t + 1], axis=0),
                        )

                    # ---- x_dst^T and Q for this group's 128 dst rows
                    xdT_ps = pp.tile([128, 2, 128], F32, tag="tp")
                    for c in range(2):
                        nc.tensor.transpose(out=xdT_ps[:, c, :],
                                            in_=xd[:, c * 128:(c + 1) * 128],
                                            identity=ident[:])
                    xdT = sp.tile([128, 2, 128], F32, tag="xdT")
                    nc.vector.tensor_copy(xdT[:], xdT_ps[:])
                    qg_ps = pp.tile([128, D], F32, tag="py")
                    for c in range(2):
                        nc.tensor.matmul(out=qg_ps[:], lhsT=xdT[:, c, :],
                                         rhs=wq[:, c, :],
                                         start=(c == 0), stop=(c == 1))
                    qg = sp.tile([128, D], F32, tag="qg")
                    nc.scalar.copy(qg[:], qg_ps[:])

                    # ---- one-hot (edge -> dst-local) in both orientations
                    oT = sp.tile([128, T, 128], F32, tag="oT")
                    for t in range(T):
                        nc.vector.tensor_tensor(
                            out=oT[:, t, :],
                            in0=dcol[:, t:t + 1].to_broadcast([128, 128]),
                            in1=iota[:], op=OP.is_equal)
                    od_ps = pp.tile([128, T, 128], F32, tag="od")
                    for t in range(T):
                        nc.tensor.transpose(out=od_ps[:, t, :], in_=oT[:, t, :],
                                            identity=ident[:])
                    od = sp.tile([128, T, 128], F32, tag="odsb")
                    nc.vector.tensor_copy(od[:], od_ps[:])

                    ae = sp.tile([128, T, H], F32, tag="ae")
                    denom_ps = pp.tile([128, H], F32, tag="den")
                    aggT_ps0 = pp.tile([128, 128], F32, tag="agg0")
                    aggT_ps1 = pp.tile([128, 128], F32, tag="agg1")
                    aggT_ps = [aggT_ps0, aggT_ps1]

                    for t in range(T):
                        # gathered src rows, transposed
                        xgT_ps = pp.tile([128, 2, 128], F32, tag="tp")
                        for c in range(2):
                            nc.tensor.transpose(out=xgT_ps[:, c, :],
                                                in_=xg[:, t, c * 128:(c + 1) * 128],
                                                identity=ident[:])
                        xgT = sp.tile([128, 2, 128], F32, tag="xgT")
                        nc.vector.tensor_copy(xgT[:], xgT_ps[:])

                        k_ps = pp3.tile([128, D], F32, tag="proj")
                        for c in range(2):
                            nc.tensor.matmul(out=k_ps[:], lhsT=xgT[:, c, :],
                                             rhs=wk[:, c, :],
                                             start=(c == 0), stop=(c == 1))
                        k_sb = sp.tile([128, D], F32, tag="k_sb")
                        nc.scalar.copy(k_sb[:], k_ps[:])
                        v_ps = pp3.tile([128, D], F32, tag="proj")
                        for c in range(2):
                            nc.tensor.matmul(out=v_ps[:], lhsT=xgT[:, c, :],
                                             rhs=wv[:, c, :],
                                             start=(c == 0), stop=(c == 1))

                        # q expanded to edges: [e, D] = od[:,t,:].T @ qg
                        qe_ps = pp3.tile([128, D], F32, tag="proj")
                        nc.tensor.matmul(out=qe_ps[:], lhsT=od[:, t, :],
                                         rhs=qg[:], start=True, stop=True)

                        # attention logits + exp
                        qkm = sp.tile([128, D], F32, tag="qkm")
                        nc.vector.tensor_tensor(out=qkm[:], in0=qe_ps[:],
                                                in1=k_sb[:], op=OP.mult)
                        attn = sp.tile([128, H], F32, tag="attn")
                        nc.vector.tensor_reduce(
                            out=attn[:],
                            in_=qkm[:].rearrange("p (h j) -> p h j", j=DH),
                            axis=mybir.AxisListType.X, op=OP.add)
                        nc.scalar.activation(ae[:, t, :], attn[:], AF.Exp)

                        # weighted V rows (unnormalized)
                        anb = sp.tile([128, H, DH], F32, tag="anb")
                        nc.vector.tensor_copy(
                            anb[:], ae[:, t, :, None].to_broadcast([128, H, DH]))
                        wV = sp.tile([128, D], F32, tag="wV")
                        nc.vector.tensor_tensor(
                            out=wV[:].rearrange("p (h j) -> p h j", j=DH),
                            in0=anb[:],
                            in1=v_ps[:].rearrange("p (h j) -> p h j", j=DH),
                            op=OP.mult)

                        # segment sums via one-hot matmuls
                        nc.tensor.matmul(out=denom_ps[:], lhsT=oT[:, t, :],
                                         rhs=ae[:, t, :],
                                         start=(t == 0), stop=(t == T - 1))
                        for c in range(2):
                            nc.tensor.matmul(out=aggT_ps[c][:],
                                             lhsT=wV[:, c * 128:(c + 1) * 128],
                                             rhs=oT[:, t, :],
                                             start=(t == 0), stop=(t == T - 1))

                    # ---- normalize aggregate:  aggT[f, d] /= denom[d, head(f)]
                    rec = sp.tile([128, H], F32, tag="rec")
                    nc.vector.tensor_scalar(out=rec[:], in0=denom_ps[:],
                                            scalar1=1e-30, scalar2=None,
                                            op0=OP.add)
                    nc.vector.reciprocal(rec[:], rec[:])
                    reb = sp.tile([128, H, DH], F32, tag="reb")
                    nc.vector.tensor_copy(
                        reb[:], rec[:, :, None].to_broadcast([128, H, DH]))
                    R_ps = pp.tile([128, 2, 128], F32, tag="tp")
                    for c in range(2):
                        nc.tensor.transpose(
                            out=R_ps[:, c, :],
                            in_=reb[:].rearrange("p h j -> p (h j)")[:, c * 128:(c + 1) * 128],
                            identity=ident[:])
                    Rsb = sp.tile([128, 2, 128], F32, tag="Rsb")
                    nc.vector.tensor_copy(Rsb[:], R_ps[:])
                    aggT = sp.tile([128, 2, 128], F32, tag="aggTsb")
                    for c in range(2):
                        nc.vector.tensor_tensor(out=aggT[:, c, :],
                                                in0=aggT_ps[c][:],
                                                in1=Rsb[:, c, :], op=OP.mult)

                    # ---- y = aggT.T @ Wmsg + x @ Wskip + bskip ; relu; LN
                    y_ps = pp.tile([128, D], F32, tag="py")
                    nc.tensor.matmul(out=y_ps[:], lhsT=ones1[:], rhs=bskip[:],
                                     start=True, stop=False)
                    for c in range(2):
                        nc.tensor.matmul(out=y_ps[:], lhsT=aggT[:, c, :],
                                         rhs=wmsg[:, c, :], start=False, stop=False)
                    for c in range(2):
                        nc.tensor.matmul(out=y_ps[:], lhsT=xdT[:, c, :],
                                         rhs=wskip[:, c, :], start=False,
                                         stop=(c == 1))
                    zr = sp.tile([128, D], F32, tag="zr")
                    nc.scalar.activation(zr[:], y_ps[:], AF.Relu)

                    msum = sp.tile([128, 1], F32, tag="msum")
                    nc.vector.reduce_sum(out=msum[:], in_=zr[:],
                                         axis=mybir.AxisListType.X)
                    mcol = sp.tile([128, 1], F32, tag="mcol")
                    nc.vector.tensor_scalar(out=mcol[:], in0=msum[:],
                                            scalar1=1.0 / D, scalar2=None,
                                            op0=OP.mult)
                    xc = sp.tile([128, D], F32, tag="xc")
                    nc.vector.tensor_scalar(out=xc[:], in0=zr[:],
                                            scalar1=mcol[:, :1], scalar2=None,
                                            op0=OP.subtract)
                    sqd = sp.tile([128, D], F32, tag="sqd")
                    vs = sp.tile([128, 1], F32, tag="vs")
                    nc.scalar.activation(sqd[:], xc[:], AF.Square,
                                         accum_out=vs[:, :1])
                    varp = sp.tile([128, 1], F32, tag="varp")
                    nc.vector.tensor_scalar(out=varp[:], in0=vs[:],
                                            scalar1=1.0 / D, scalar2=EPS,
                                            op0=OP.mult, op1=OP.add)
                    rv = sp.tile([128, 1], F32, tag="rv")
                    nc.vector.reciprocal(rv[:], varp[:])
                    rstd = sp.tile([128, 1], F32, tag="rstd")
                    nc.scalar.activation(rstd[:], rv[:], AF.Sqrt)
                    xn = sp.tile([128, D], F32, tag="xn")
                    nc.vector.tensor_scalar(out=xn[:], in0=xc[:],
                                            scalar1=rstd[:, :1], scalar2=None,
                                            op0=OP.mult)
                    xg2 = sp.tile([128, D], F32, tag="xg2")
                    nc.vector.tensor_tensor(out=xg2[:], in0=xn[:], in1=gln[:],
                                            op=OP.mult)
                    fin = sp.tile([128, D], F32, tag="fin")
                    nc.vector.tensor_tensor(out=fin[:], in0=xg2[:], in1=bln[:],
                                            op=OP.add)
                    nc.sync.dma_start(out=outd[ds(g, 128), :], in_=fin[:])
    nc.compile()
    return nc


# ------------------------------------------------------------------- driver

def _sigmoid(x):
    return 1.0 / (1.0 + np.exp(-x))


TRACE = False
LAST = None


def kernel(x_a, x_b, Wq_a, Wk_a, Wv_a, Wq_b, Wk_b, Wv_b,
           Wskip_a_w, Wskip_a_b, Wskip_b_w, Wskip_b_b,
           g_a, b_a, g_b, b_b, mu_ab, Wmsg_ab, mu_ba, Wmsg_ba,
           ei_ab, ei_ba):
    from concourse.bass_utils import run_bass_kernel_spmd

    x_a = np.asarray(x_a, np.float32)
    x_b = np.asarray(x_b, np.float32)
    SCALE = DH ** -0.5

    cap = max(_edge_capacity(np.asarray(ei_ab[1])),
              _edge_capacity(np.asarray(ei_ba[1])))
    T = max(1, -(-cap // 128))

    src_ab, dstl_ab = _pack_edges(np.asarray(ei_ab[0]), np.asarray(ei_ab[1]), T)
    src_ba, dstl_ba = _pack_edges(np.asarray(ei_ba[0]), np.asarray(ei_ba[1]), T)

    xa_dst = _shard_rows(x_a)
    xb_dst = _shard_rows(x_b)

    def fold_q(Wq, mu):
        s = (SCALE * _sigmoid(np.asarray(mu, np.float64))).astype(np.float32)
        return (np.asarray(Wq, np.float32) * np.repeat(s, DH)[None, :]).copy()

    bc = lambda v: np.broadcast_to(np.asarray(v, np.float32)[None, :], (128, D)).copy()
    iota_row = np.broadcast_to(np.arange(128, dtype=np.float32)[None, :],
                               (128, 128)).copy()

    shared = {
        "xa_full": x_a, "xb_full": x_b, "iota_row": iota_row,
        # relation ab: src a -> dst b (out_b)
        "wq_ab": fold_q(Wq_b, mu_ab), "wk_ab": np.asarray(Wk_a, np.float32),
        "wv_ab": np.asarray(Wv_a, np.float32),
        "wmsg_ab": np.asarray(Wmsg_ab, np.float32),
        "wskip_ab": np.asarray(Wskip_b_w, np.float32),
        "bskip_ab": np.asarray(Wskip_b_b, np.float32).reshape(1, D),
        "gln_ab": bc(g_b), "bln_ab": bc(b_b),
        # relation ba: src b -> dst a (out_a)
        "wq_ba": fold_q(Wq_a, mu_ba), "wk_ba": np.asarray(Wk_b, np.float32),
        "wv_ba": np.asarray(Wv_b, np.float32),
        "wmsg_ba": np.asarray(Wmsg_ba, np.float32),
        "wskip_ba": np.asarray(Wskip_a_w, np.float32),
        "bskip_ba": np.asarray(Wskip_a_b, np.float32).reshape(1, D),
        "gln_ba": bc(g_a), "bln_ba": bc(b_a),
    }
    in_maps = []
    for m in range(M):
        im = dict(shared)
        im["xa_dst"] = xa_dst[m]
        im["xb_dst"] = xb_dst[m]
        im["src_ab"] = src_ab[m]
        im["dstl_ab"] = dstl_ab[m]
        im["src_ba"] = src_ba[m]
        im["dstl_ba"] = dstl_ba[m]
        in_maps.append(im)

    nc = build_program(T)
    res = run_bass_kernel_spmd(nc, in_maps, list(range(M)), trace=TRACE)
    global LAST
    LAST = res
    out_a = np.empty((N, D), np.float32)
    out_b = np.empty((N, D), np.float32)
    for m in range(M):
        out_b[m * NSH:(m + 1) * NSH] = res.results[m]["out_ab"][:NSH]
        out_a[m * NSH:(m + 1) * NSH] = res.results[m]["out_ba"][:NSH]
    return out_a, out_b



# revision 10
# speedup vs baseline: 3.0597x; 3.0597x over previous
"""HGT layer (2 node types, 2 relations) on 8 Trainium2 cores.

Strategy (dst-sharded, single fused pass, bf16 datapath):
  - Each core owns 12500 destination nodes of each type. Edges are
    partitioned by destination shard on the host and sorted into groups
    of 128 consecutive destination nodes, padded to a fixed per-group
    edge capacity C = T*128 (slot s -> SBUF partition s%128, col s//128).
  - Host passes bf16 copies of x (full, for the per-edge source gather)
    and of all weights.  SCALE * sigmoid(mu_h) is folded into Wq.
  - Per group the kernel: gathers source rows with ONE batched indirect
    DMA; projects [K|V] with a single 512-wide bf16 matmul per 128-chunk;
    projects [Q|skip] the same way from the group's dst rows; expands Q
    to edges through a one-hot (dst-local x edge) matmul; computes
    per-edge logits (DVE mult + per-head reduce), exponentiates (ACT);
    and aggregates numerator and softmax denominator with ONE accumulating
    matmul per edge column:  agg[d, 0:256 | 256:264] += oT[t].T @ [a*V | a].
    Normalization, Wmsg mixing, skip+bias, relu and layernorm are fused
    in the same group iteration.  LayerNorm's rstd uses exp(-0.5*ln(var))
    so every ACT call stays inside the natural_log_exp_and_others table
    set (no ACT table reloads).
  - Fully static unroll (no For_i back-edge barriers) so groups pipeline
    across engines.
"""

import numpy as np
import ml_dtypes

import concourse.bacc as bacc
import concourse.bass as bass
import concourse.mybir as mybir
import concourse.tile as tile
from concourse.bass import ds
from concourse.masks import make_identity

N = 100000
D = 256
H = 8
DH = 32
M = 8            # cores
NSH = N // M     # 12500 dst rows per core per type
G = 98           # dst groups of 128 per core (98*128 = 12544)
NPAD = G * 128   # 12544
EPS = 1e-5
F32 = mybir.dt.float32
BF16 = mybir.dt.bfloat16
I32 = mybir.dt.int32
AF = mybir.ActivationFunctionType
OP = mybir.AluOpType
BF = ml_dtypes.bfloat16


# ----------------------------------------------------------------- host prep

def _pack_edges(src, dst, T):
    """Partition edges by dst shard, group by 128 consecutive dsts, pad to
    T*128 slots per group.  Returns src_idx [M, 128, G, T] int32 and
    dstl [M, 128, G, T] float32 (dst-local-in-group; 999.0 for padding).
    Slot s of group g maps to SBUF partition p = s % 128, column t = s // 128."""
    order = np.argsort(dst, kind="stable")
    s_sorted = src[order].astype(np.int64)
    d_sorted = dst[order].astype(np.int64)

    core = d_sorted // NSH
    local = d_sorted - core * NSH
    grp = local // 128
    dloc = local - grp * 128
    key = core * G + grp
    first = np.r_[0, np.flatnonzero(np.diff(key)) + 1]
    starts = np.zeros(len(key), dtype=np.int64)
    starts[first] = first
    starts = np.maximum.accumulate(starts)
    slot = np.arange(len(key), dtype=np.int64) - starts

    maxslot = int(slot.max()) if len(slot) else 0
    assert maxslot < T * 128, f"edge capacity exceeded: {maxslot + 1} > {T * 128}"

    src_arr = np.zeros((M * G, T * 128), dtype=np.int32)
    dst_arr = np.full((M * G, T * 128), 999.0, dtype=np.float32)
    src_arr[key, slot] = s_sorted
    dst_arr[key, slot] = dloc
    # [M*G, T*128] -> [M, G, T, 128] -> [M, 128, G, T]
    src_arr = src_arr.reshape(M, G, T, 128).transpose(0, 3, 1, 2)
    dst_arr = dst_arr.reshape(M, G, T, 128).transpose(0, 3, 1, 2)
    return np.ascontiguousarray(src_arr), np.ascontiguousarray(dst_arr)


def _edge_capacity(dst):
    d = np.sort(dst.astype(np.int64))
    core = d // NSH
    grp = (d - core * NSH) // 128
    key = core * G + grp
    _, counts = np.unique(key, return_counts=True)
    return int(counts.max())


def _shard_rows_bf16(x):
    """[N, D] f32 -> [M, 128, G, D] bf16 (partition-major groups), zero pad."""
    out = np.zeros((M, NPAD, D), dtype=BF)
    for m in range(M):
        out[m, :NSH] = x[m * NSH:(m + 1) * NSH].astype(BF)
    # [M, G, 128, D] -> [M, 128, G, D]
    return np.ascontiguousarray(
        out.reshape(M, G, 128, D).transpose(0, 2, 1, 3))


# ------------------------------------------------------------- bass program

DEBUG = False


def build_program(T, ln_trivial):
    nc = bacc.Bacc("TRN2", target_bir_lowering=False, debug=False)

    def drt(name, shape, dtype=F32, kind="ExternalInput"):
        return nc.dram_tensor(name, shape, dtype, kind=kind)

    dbg = {}
    if DEBUG:
        for nm, shape, dt in [
            ("dbg_xg", [128, T, D], F32), ("dbg_oT", [128, T, 128], F32),
            ("dbg_xdT", [128, 2, 128], F32), ("dbg_kv0", [128, 2 * D], F32),
            ("dbg_qe0", [128, D], F32), ("dbg_attn0", [128, H], F32),
            ("dbg_agg", [128, D + H], F32), ("dbg_od", [128, T, 128], F32),
            ("dbg_qg", [128, D], F32), ("dbg_skip", [128, D], F32),
        ]:
            dbg[nm] = drt(nm, shape, dt, kind="ExternalOutput")

    xa_full = drt("xa_full", [N, D], BF16)
    xb_full = drt("xb_full", [N, D], BF16)
    iota_row = drt("iota_row", [128, 128], BF16)

    rels = []
    for r in ("ab", "ba"):
        rels.append(dict(
            name=r,
            edi=drt(f"edi_{r}", [128, G, T], I32),       # src indices
            edf=drt(f"edf_{r}", [128, G, T]),            # dst-local (999 pad)
            xdst=drt(f"xdst_{r}", [128, G, D], BF16),    # dst rows, bf16
            wqskip=drt(f"wqskip_{r}", [D, 2 * D], BF16),  # [Wq | Wskip]
            wkv=drt(f"wkv_{r}", [D, 2 * D], BF16),        # [Wk | Wv]
            wmsg=drt(f"wmsg_{r}", [D, D], BF16),
            bskip=drt(f"bskip_{r}", [1, D], BF16),
            out=drt(f"out_{r}", [NPAD, D], kind="ExternalOutput"),
        ))
        if not ln_trivial:
            rels[-1]["gln"] = drt(f"gln_{r}", [128, D])
            rels[-1]["bln"] = drt(f"bln_{r}", [128, D])
    rels[0]["xfull"] = xa_full   # ab: src type a -> dst type b
    rels[1]["xfull"] = xb_full

    with tile.TileContext(nc) as tc:
        with (
            tc.tile_pool(name="const", bufs=1) as cp,
            tc.tile_pool(name="sbuf", bufs=2) as sp,
            tc.tile_pool(name="sb3", bufs=3) as s3,
            tc.tile_pool(name="ptp", bufs=2, space="PSUM") as ptp,
            tc.tile_pool(name="pkv", bufs=2, space="PSUM") as pkv,
            tc.tile_pool(name="pqe", bufs=2, space="PSUM") as pqe,
            tc.tile_pool(name="pqs", bufs=1, space="PSUM") as pqs,
            tc.tile_pool(name="pagg", bufs=1, space="PSUM") as pagg,
        ):
            def dump(nm, ap, shape):
                tmp = cp.tile(shape, F32, tag=nm)
                nc.vector.tensor_copy(tmp[:], ap)
                nc.sync.dma_start(out=dbg[nm][:], in_=tmp[:])

            ident = cp.tile([128, 128], BF16)
            make_identity(nc, ident[:])
            iota = cp.tile([128, 128], BF16)
            nc.sync.dma_start(out=iota[:], in_=iota_row[:])
            ones1 = cp.tile([1, 128], BF16)
            nc.gpsimd.memset(ones1[:], 1.0)
            magic = cp.tile([128, 1], I32)
            nc.gpsimd.memset(magic[:], 0x5F3759DF)

            for rel in rels:
                r = rel["name"]
                # --- per-relation static data
                wqskip = cp.tile([128, 2, 2 * D], BF16, tag="wqskip")
                wkv = cp.tile([128, 2, 2 * D], BF16, tag="wkv")
                wmsg = cp.tile([128, 2, D], BF16, tag="wmsg")
                for c in range(2):
                    nc.sync.dma_start(out=wqskip[:, c, :],
                                      in_=rel["wqskip"][c * 128:(c + 1) * 128, :])
                    nc.sync.dma_start(out=wkv[:, c, :],
                                      in_=rel["wkv"][c * 128:(c + 1) * 128, :])
                    nc.sync.dma_start(out=wmsg[:, c, :],
                                      in_=rel["wmsg"][c * 128:(c + 1) * 128, :])
                bskip = cp.tile([1, D], BF16, tag="bskip")
                nc.sync.dma_start(out=bskip[:], in_=rel["bskip"][:])
                if not ln_trivial:
                    gln = cp.tile([128, D], F32, tag="gln")
                    bln = cp.tile([128, D], F32, tag="bln")
                    nc.sync.dma_start(out=gln[:], in_=rel["gln"][:])
                    nc.sync.dma_start(out=bln[:], in_=rel["bln"][:])
                edi = cp.tile([128, G, T], I32, tag="edi")
                nc.sync.dma_start(out=edi[:], in_=rel["edi"][:])
                edf = cp.tile([128, G, T], F32, tag="edf")
                nc.sync.dma_start(out=edf[:], in_=rel["edf"][:])
                xdall = cp.tile([128, G, D], BF16, tag="xdall")
                for q in range(0, G, 14):
                    qe_ = min(q + 14, G)
                    nc.sync.dma_start(out=xdall[:, q:qe_, :],
                                      in_=rel["xdst"][:, q:qe_, :])

                xfull, outd = rel["xfull"], rel["out"]

                dbg_on = DEBUG and rel["name"] == "ab"
                for g in range(G):
                    dbg_g = dbg_on and g == 0
                    # ---- source-row gathers (per edge column; SWDGE)
                    xg = s3.tile([128, T, D], BF16, tag="xg")
                    for t in range(T):
                        nc.gpsimd.indirect_dma_start(
                            out=xg[:, t, :], out_offset=None,
                            in_=xfull[:],
                            in_offset=bass.IndirectOffsetOnAxis(
                                ap=edi[:, g, t:t + 1], axis=0),
                        )

                    if dbg_g:
                        dump("dbg_xg", xg[:], [128, T, D])
                    # ---- one-hot, both orientations
                    oT = sp.tile([128, T, 128], BF16, tag="oT")
                    nc.vector.tensor_tensor(
                        out=oT[:],
                        in0=edf[:, g, :, None].to_broadcast([128, T, 128]),
                        in1=iota[:, None, :].to_broadcast([128, T, 128]),
                        op=OP.is_equal)
                    od_ps = ptp.tile([128, T, 128], BF16, tag="tp")
                    for t in range(T):
                        nc.tensor.transpose(out=od_ps[:, t, :], in_=oT[:, t, :],
                                            identity=ident[:])
                    od = sp.tile([128, T, 128], BF16, tag="od")
                    nc.vector.tensor_copy(od[:], od_ps[:])
                    if dbg_g:
                        dump("dbg_oT", oT[:], [128, T, 128])
                        dump("dbg_od", od[:], [128, T, 128])

                    # ---- dst rows: transpose, then [Q | skip] projection
                    xdT_ps = ptp.tile([128, 2, 128], BF16, tag="tp")
                    for c in range(2):
                        nc.tensor.transpose(out=xdT_ps[:, c, :],
                                            in_=xdall[:, g, c * 128:(c + 1) * 128],
                                            identity=ident[:])
                    xdT = sp.tile([128, 2, 128], BF16, tag="xdT")
                    nc.vector.tensor_copy(xdT[:], xdT_ps[:])
                    q_ps = pqe.tile([128, D], F32, tag="qe")
                    for c in range(2):
                        nc.tensor.matmul(out=q_ps[:], lhsT=xdT[:, c, :],
                                         rhs=wqskip[:, c, :D],
                                         start=(c == 0), stop=(c == 1))
                    qg = sp.tile([128, D], BF16, tag="qg")
                    nc.vector.tensor_copy(qg[:], q_ps[:])
                    if dbg_g:
                        dump("dbg_xdT", xdT[:], [128, 2, 128])
                        dump("dbg_qg", qg[:], [128, D])

                    agg_ps = pagg.tile([128, D + H], F32, tag="agg")

                    for t in range(T):
                        # gathered source rows, transposed
                        xgT_ps = ptp.tile([128, 2, 128], BF16, tag="tp")
                        for c in range(2):
                            nc.tensor.transpose(out=xgT_ps[:, c, :],
                                                in_=xg[:, t, c * 128:(c + 1) * 128],
                                                identity=ident[:])
                        xgT = sp.tile([128, 2, 128], BF16, tag="xgT")
                        nc.vector.tensor_copy(xgT[:], xgT_ps[:])

                        # [K | V] projection (512-wide rhs)
                        kv_ps = pkv.tile([128, 2 * D], F32, tag="kv")
                        for c in range(2):
                            nc.tensor.matmul(out=kv_ps[:], lhsT=xgT[:, c, :],
                                             rhs=wkv[:, c, :],
                                             start=(c == 0), stop=(c == 1))

                        # q expanded to edges
                        qe_ps = pqe.tile([128, D], F32, tag="qe")
                        nc.tensor.matmul(out=qe_ps[:], lhsT=od[:, t, :],
                                         rhs=qg[:], start=True, stop=True)
                        qe_sb = sp.tile([128, D], F32, tag="qe_sb")
                        nc.vector.tensor_copy(qe_sb[:], qe_ps[:])
                        if dbg_g and t == 0:
                            dump("dbg_kv0", kv_ps[:], [128, 2 * D])
                            dump("dbg_qe0", qe_sb[:], [128, D])

                        # logits -> exp -> weighted V  (a | aV share one tile)
                        qkm = sp.tile([128, D], F32, tag="qkm")
                        nc.vector.tensor_tensor(out=qkm[:], in0=qe_sb[:],
                                                in1=kv_ps[:, :D], op=OP.mult)
                        attn = sp.tile([128, H], F32, tag="attn")
                        nc.vector.tensor_reduce(
                            out=attn[:],
                            in_=qkm[:].rearrange("p (h j) -> p h j", j=DH),
                            axis=mybir.AxisListType.X, op=OP.add)
                        if dbg_g and t == 0:
                            dump("dbg_attn0", attn[:], [128, H])
                        wVae = sp.tile([128, D + H], BF16, tag="wVae")
                        nc.scalar.activation(wVae[:, D:], attn[:], AF.Exp)
                        nc.vector.tensor_tensor(
                            out=wVae[:, :D].rearrange("p (h j) -> p h j", j=DH),
                            in0=wVae[:, D:, None].to_broadcast([128, H, DH]),
                            in1=kv_ps[:, D:].rearrange("p (h j) -> p h j", j=DH),
                            op=OP.mult)

                        # numerator + denominator in one accumulating matmul
                        nc.tensor.matmul(out=agg_ps[:], lhsT=oT[:, t, :],
                                         rhs=wVae[:],
                                         start=(t == 0), stop=(t == T - 1))

                    if dbg_g:
                        dump("dbg_agg", agg_ps[:], [128, D + H])
                    # ---- normalize, transpose agg for the Wmsg matmul
                    rec = sp.tile([128, H], F32, tag="rec")
                    nc.vector.tensor_scalar(out=rec[:], in0=agg_ps[:, D:],
                                            scalar1=1e-30, scalar2=None,
                                            op0=OP.add)
                    nc.vector.reciprocal(rec[:], rec[:])
                    aggn = sp.tile([128, D], BF16, tag="aggn")
                    nc.vector.tensor_tensor(
                        out=aggn[:].rearrange("p (h j) -> p h j", j=DH),
                        in0=agg_ps[:, :D].rearrange("p (h j) -> p h j", j=DH),
                        in1=rec[:, :, None].to_broadcast([128, H, DH]),
                        op=OP.mult)
                    aggT_ps = ptp.tile([128, 2, 128], BF16, tag="tp")
                    for c in range(2):
                        nc.tensor.transpose(out=aggT_ps[:, c, :],
                                            in_=aggn[:, c * 128:(c + 1) * 128],
                                            identity=ident[:])
                    aggT = sp.tile([128, 2, 128], BF16, tag="aggT")
                    nc.vector.tensor_copy(aggT[:], aggT_ps[:])
                    skip_ps = pqs.tile([128, D], F32, tag="qskip")
                    for c in range(2):
                        nc.tensor.matmul(out=skip_ps[:], lhsT=xdT[:, c, :],
                                         rhs=wqskip[:, c, D:],
                                         start=(c == 0), stop=False)
                    nc.tensor.matmul(out=skip_ps[:], lhsT=ones1[:],
                                     rhs=bskip[:], start=False, stop=False)
                    for c in range(2):
                        nc.tensor.matmul(out=skip_ps[:], lhsT=aggT[:, c, :],
                                         rhs=wmsg[:, c, :], start=False,
                                         stop=(c == 1))

                    # ---- relu + layernorm (ACT heavy; ln/exp table set only)
                    if dbg_g:
                        dump("dbg_skip", skip_ps[:], [128, D])
                    zr = sp.tile([128, D], F32, tag="zr")
                    msum = sp.tile([128, 1], F32, tag="msum")
                    nc.scalar.activation(zr[:], skip_ps[:], AF.Relu,
                                         accum_out=msum[:, :1])
                    mb = sp.tile([128, 1], F32, tag="mb")
                    nc.vector.tensor_scalar(out=mb[:], in0=msum[:],
                                            scalar1=-1.0 / D, scalar2=None,
                                            op0=OP.mult)
                    xc = sp.tile([128, D], F32, tag="xc")
                    sq = sp.tile([128, D], F32, tag="sq")
                    vs = sp.tile([128, 1], F32, tag="vs")
                    nc.scalar.activation(xc[:], zr[:], AF.Identity,
                                         bias=mb[:, :1])
                    nc.scalar.activation(sq[:], xc[:], AF.Square,
                                         accum_out=vs[:, :1])
                    varp = sp.tile([128, 1], F32, tag="varp")
                    nc.vector.tensor_scalar(out=varp[:], in0=vs[:],
                                            scalar1=1.0 / D, scalar2=EPS,
                                            op0=OP.mult, op1=OP.add)
                    # rstd = 1/sqrt(varp) via magic-constant + 2 Newton steps
                    ji = sp.tile([128, 1], I32, tag="ji")
                    nc.vector.tensor_scalar(out=ji[:], in0=varp[:].bitcast(I32),
                                            scalar1=1, scalar2=None,
                                            op0=OP.logical_shift_right)
                    rstd = sp.tile([128, 1], F32, tag="rstd")
                    nc.vector.tensor_tensor(out=rstd[:].bitcast(I32),
                                            in0=magic[:], in1=ji[:],
                                            op=OP.subtract)
                    nt = sp.tile([128, 1], F32, tag="nt")
                    for _ in range(2):
                        nc.vector.tensor_tensor(out=nt[:], in0=rstd[:],
                                                in1=rstd[:], op=OP.mult)
                        nc.vector.tensor_tensor(out=nt[:], in0=nt[:],
                                                in1=varp[:], op=OP.mult)
                        nc.vector.tensor_scalar(out=nt[:], in0=nt[:],
                                                scalar1=-0.5, scalar2=1.5,
                                                op0=OP.mult, op1=OP.add)
                        nc.vector.tensor_tensor(out=rstd[:], in0=rstd[:],
                                                in1=nt[:], op=OP.mult)
                    fin = sp.tile([128, D], F32, tag="fin")
                    nc.scalar.activation(fin[:], xc[:], AF.Identity,
                                         scale=rstd[:, :1])
                    if not ln_trivial:
                        fin2 = sp.tile([128, D], F32, tag="fin2")
                        nc.vector.tensor_tensor(out=fin2[:], in0=fin[:],
                                                in1=gln[:], op=OP.mult)
                        nc.vector.tensor_tensor(out=fin2[:], in0=fin2[:],
                                                in1=bln[:], op=OP.add)
                        fin = fin2
                    nc.sync.dma_start(out=outd[ds(g * 128, 128), :], in_=fin[:])
    nc.compile()
    return nc


# ------------------------------------------------------------------- driver

def _sigmoid(x):
    return 1.0 / (1.0 + np.exp(-x))


TRACE = False
LAST = None


def kernel(x_a, x_b, Wq_a, Wk_a, Wv_a, Wq_b, Wk_b, Wv_b,
           Wskip_a_w, Wskip_a_b, Wskip_b_w, Wskip_b_b,
           g_a, b_a, g_b, b_b, mu_ab, Wmsg_ab, mu_ba, Wmsg_ba,
           ei_ab, ei_ba):
    from concourse.bass_utils import run_bass_kernel_spmd

    x_a = np.asarray(x_a, np.float32)
    x_b = np.asarray(x_b, np.float32)
    SCALE = DH ** -0.5

    cap = max(_edge_capacity(np.asarray(ei_ab[1])),
              _edge_capacity(np.asarray(ei_ba[1])))
    T = max(1, -(-cap // 128))

    edi_ab, edf_ab = _pack_edges(np.asarray(ei_ab[0]), np.asarray(ei_ab[1]), T)
    edi_ba, edf_ba = _pack_edges(np.asarray(ei_ba[0]), np.asarray(ei_ba[1]), T)

    xa_dst = _shard_rows_bf16(x_a)
    xb_dst = _shard_rows_bf16(x_b)

    def fold_q(Wq, mu):
        s = (SCALE * _sigmoid(np.asarray(mu, np.float64))).astype(np.float32)
        return np.asarray(Wq, np.float32) * np.repeat(s, DH)[None, :]

    def cat_bf(a, b):
        return np.ascontiguousarray(
            np.concatenate([np.asarray(a, np.float32),
                            np.asarray(b, np.float32)], axis=1).astype(BF))

    bc = lambda v: np.broadcast_to(np.asarray(v, np.float32)[None, :], (128, D)).copy()
    iota_row = np.broadcast_to(np.arange(128, dtype=np.float32)[None, :],
                               (128, 128)).astype(BF).copy()

    ln_trivial = bool(
        np.all(np.asarray(g_a) == 1.0) and np.all(np.asarray(b_a) == 0.0)
        and np.all(np.asarray(g_b) == 1.0) and np.all(np.asarray(b_b) == 0.0))

    shared = {
        "xa_full": x_a.astype(BF), "xb_full": x_b.astype(BF),
        "iota_row": iota_row,
        # relation ab: src a -> dst b (out_b)
        "wqskip_ab": cat_bf(fold_q(Wq_b, mu_ab), Wskip_b_w),
        "wkv_ab": cat_bf(Wk_a, Wv_a),
        "wmsg_ab": np.asarray(Wmsg_ab, np.float32).astype(BF),
        "bskip_ab": np.asarray(Wskip_b_b, np.float32).astype(BF).reshape(1, D),
        # relation ba: src b -> dst a (out_a)
        "wqskip_ba": cat_bf(fold_q(Wq_a, mu_ba), Wskip_a_w),
        "wkv_ba": cat_bf(Wk_b, Wv_b),
        "wmsg_ba": np.asarray(Wmsg_ba, np.float32).astype(BF),
        "bskip_ba": np.asarray(Wskip_a_b, np.float32).astype(BF).reshape(1, D),
    }
    if not ln_trivial:
        shared.update({
            "gln_ab": bc(g_b), "bln_ab": bc(b_b),
            "gln_ba": bc(g_a), "bln_ba": bc(b_a),
        })
    in_maps = []
    for m in range(M):
        im = dict(shared)
        im["xdst_ab"] = xb_dst[m]     # dst of ab is type b
        im["xdst_ba"] = xa_dst[m]
        im["edi_ab"] = edi_ab[m]
        im["edf_ab"] = edf_ab[m]
        im["edi_ba"] = edi_ba[m]
        im["edf_ba"] = edf_ba[m]
        if ln_trivial:
            for k in ("gln_ab", "bln_ab", "gln_ba", "bln_ba"):
                im.pop(k, None)
        in_maps.append(im)

    nc = build_program(T, ln_trivial)
    res = run_bass_kernel_spmd(nc, in_maps, list(range(M)), trace=TRACE)
    global LAST
    LAST = res
    out_a = np.empty((N, D), np.float32)
    out_b = np.empty((N, D), np.float32)
    for m in range(M):
        out_b[m * NSH:(m + 1) * NSH] = res.results[m]["out_ab"][:NSH]
        out_a[m * NSH:(m + 1) * NSH] = res.results[m]["out_ba"][:NSH]
    return out_a, out_b


# revision 14
# speedup vs baseline: 4.4270x; 1.4469x over previous
"""HGT layer (2 node types, 2 relations) on 8 Trainium2 cores.

Strategy (dst-sharded, single fused pass, bf16 datapath):
  - Each core owns 12500 destination nodes of each type. Edges are
    partitioned by destination shard on the host and sorted into groups
    of 128 consecutive destination nodes, padded to a fixed per-group
    edge capacity C = T*128 (slot s -> partition s%128, column s//128).
  - Groups are banded into 4 segments of ~25 groups; for each segment the
    host builds a deduplicated source-row table (bf16) plus int16 local
    indices, so the per-group source gather is ONE transposed dma_gather
    (InstDMAGatherAnt) that lands K-contraction-major tiles directly --
    no on-chip transposes of gathered rows.
  - The host also supplies, per group: pre-transposed dst rows (for the
    Q/skip matmul lhsT) and both one-hot orientations (edge x dst-local).
  - Per group: [K|V] projection with one 512-wide bf16 matmul per
    128-chunk; Q projection; Q expanded to edges via the one-hot matmul;
    per-edge logits (DVE mult reading both PSUM operands + per-head
    reduce on GpSimd), exp (ACT); numerator and softmax denominator in
    one accumulating matmul per edge column:
       agg[d, 0:256 | 256:264] += oT[t].T @ [a*V | a].
    Normalization, Wmsg mixing, skip+bias, relu and layernorm fused in
    the same iteration.  LayerNorm uses var = E[z^2] - m^2 and a
    magic-constant Newton rsqrt on DVE, so the Scalar engine needs only
    one activation-table set (exp) for the whole kernel.
  - Fully static unroll (no For_i back-edge barriers); PSUM pools sized
    so consecutive groups overlap (kv/qe/agg double-buffered).
"""

import numpy as np
import ml_dtypes

import concourse.bacc as bacc
import concourse.bass as bass
import concourse.mybir as mybir
import concourse.tile as tile
from concourse.bass import ds
from concourse.masks import make_identity

N = 100000
D = 256
H = 8
DH = 32
M = 8            # cores
NSH = N // M     # 12500 dst rows per core per type
G = 98           # dst groups of 128 per core (98*128 = 12544)
NPAD = G * 128   # 12544
NSEG = 4         # source-table segments (groups g//25)
SEGG = 25        # groups per segment
U = 12288        # table rows per segment (>= max unique sources + pad)
EPS = 1e-5
F32 = mybir.dt.float32
BF16 = mybir.dt.bfloat16
I32 = mybir.dt.int32
I16 = mybir.dt.int16
AF = mybir.ActivationFunctionType
OP = mybir.AluOpType
BF = ml_dtypes.bfloat16


# ----------------------------------------------------------------- host prep

def _pack_edges(src, dst, T):
    """Partition edges by dst shard into groups of 128 dsts with T*128 slots.
    Returns src_idx [M, G, T, 128] int64 and dloc [M, G, T, 128] int64
    (slot (t, e): partition e, column t; dloc 999 for padding, src 0)."""
    order = np.argsort(dst, kind="stable")
    s_sorted = src[order].astype(np.int64)
    d_sorted = dst[order].astype(np.int64)

    core = d_sorted // NSH
    local = d_sorted - core * NSH
    grp = local // 128
    dloc = local - grp * 128
    key = core * G + grp
    first = np.r_[0, np.flatnonzero(np.diff(key)) + 1]
    starts = np.zeros(len(key), dtype=np.int64)
    starts[first] = first
    starts = np.maximum.accumulate(starts)
    slot = np.arange(len(key), dtype=np.int64) - starts

    maxslot = int(slot.max()) if len(slot) else 0
    assert maxslot < T * 128, f"edge capacity exceeded: {maxslot + 1} > {T * 128}"

    src_arr = np.zeros((M * G, T * 128), dtype=np.int64)
    dst_arr = np.full((M * G, T * 128), 999, dtype=np.int64)
    # slot s -> (t = s // 128, e = s % 128): flat index t*128+e = s
    src_arr[key, slot] = s_sorted
    dst_arr[key, slot] = dloc
    return (src_arr.reshape(M, G, T, 128), dst_arr.reshape(M, G, T, 128))


def _edge_capacity(dst):
    d = np.sort(dst.astype(np.int64))
    core = d // NSH
    grp = (d - core * NSH) // 128
    key = core * G + grp
    _, counts = np.unique(key, return_counts=True)
    return int(counts.max())


def _host_tables(src_idx, x_bf, T):
    """Per (core, segment) deduplicated source tables + int16 indices.
    Returns tabs [M, NSEG, U, D] bf16 and idx16 [M, 128, G, T*128//16] i16."""
    tabs = np.zeros((M, NSEG, U, D), dtype=BF)
    idxw = np.zeros((M, 128, G, T * 128 // 16), dtype=np.int16)
    for m in range(M):
        for s in range(NSEG):
            g0, g1 = s * SEGG, min((s + 1) * SEGG, G)
            srcs = src_idx[m, g0:g1].reshape(-1)          # [(g1-g0)*T*128]
            uniq, inv = np.unique(srcs, return_inverse=True)
            assert len(uniq) <= U, f"segment table overflow: {len(uniq)} > {U}"
            tabs[m, s, :len(uniq)] = x_bf[uniq]
            inv = inv.astype(np.int16).reshape(g1 - g0, T * 128)
            # index i lives at [i % 16, i // 16]
            iw = inv.reshape(g1 - g0, T * 128 // 16, 16)
            idxw[m, :16, g0:g1] = iw.transpose(2, 0, 1)
    idxw[:, 16:] = np.tile(idxw[:, :16], (1, 7, 1, 1))
    return tabs, idxw


def _host_onehots(dloc, T):
    """[M, G, T, 128] dloc -> onehot [M, 128, G, 2*T*128] bf16:
    [:, :, g, 0:T*128] = oT (partition = edge slot), [T*128:] = od."""
    oh = np.zeros((M, 128, G, 2 * T * 128), dtype=BF)
    rng = np.arange(128)
    for m in range(M):
        # oT[e, t*128+j] = (dloc[g, t, e] == j)
        oT = (dloc[m][:, :, :, None] == rng[None, None, None, :])  # [G,T,128e,128j]
        oh[m, :, :, :T * 128] = oT.transpose(2, 0, 1, 3).reshape(128, G, T * 128)
        od = oT.transpose(3, 0, 1, 2)   # [j, G, T, e]
        oh[m, :, :, T * 128:] = od.reshape(128, G, T * 128)
    return oh


def _host_xdT(x):
    """[N, D] f32 -> [M, 128, G, 2, 128] bf16 transposed dst rows:
    [:, dmod, g, c, j] = x[m*NSH + g*128 + j, c*128 + dmod]."""
    out = np.zeros((M, NPAD, D), dtype=BF)
    for m in range(M):
        out[m, :NSH] = x[m * NSH:(m + 1) * NSH].astype(BF)
    # [M, G, 128j, 2, 128dmod] -> [M, 128dmod, G, 2, 128j]
    v = out.reshape(M, G, 128, 2, 128).transpose(0, 4, 1, 3, 2)
    return np.ascontiguousarray(v)


# ------------------------------------------------------------- bass program

DEBUG = False


def build_program(T, ln_trivial):
    nc = bacc.Bacc("TRN2", target_bir_lowering=False, debug=False)

    def drt(name, shape, dtype=F32, kind="ExternalInput"):
        return nc.dram_tensor(name, shape, dtype, kind=kind)

    TL = T * 128

    rels = []
    for r in ("ab", "ba"):
        rels.append(dict(
            name=r,
            tabs=drt(f"tabs_{r}", [NSEG, U, D], BF16),
            idx16=drt(f"idx16_{r}", [128, G, TL // 16], I16),
            onehot=drt(f"onehot_{r}", [128, G, 2 * TL], BF16),
            xdT=drt(f"xdT_{r}", [128, G, 2, 128], BF16),
            wqskip=drt(f"wqskip_{r}", [D, 2 * D], BF16),  # [Wq | Wskip]
            wkv=drt(f"wkv_{r}", [D, 2 * D], BF16),        # [Wk | Wv]
            wmsg=drt(f"wmsg_{r}", [D, D], BF16),
            bskip=drt(f"bskip_{r}", [1, D], BF16),
            out=drt(f"out_{r}", [NPAD, D], kind="ExternalOutput"),
        ))
        if not ln_trivial:
            rels[-1]["gln"] = drt(f"gln_{r}", [128, D])
            rels[-1]["bln"] = drt(f"bln_{r}", [128, D])

    with tile.TileContext(nc) as tc:
        with (
            tc.tile_pool(name="const", bufs=1) as cp,
            tc.tile_pool(name="sbuf", bufs=2) as sp,
            tc.tile_pool(name="sb3", bufs=3) as s3,
            tc.tile_pool(name="ptp", bufs=1, space="PSUM") as ptp,
            tc.tile_pool(name="pkv", bufs=2, space="PSUM") as pkv,
            tc.tile_pool(name="pqe", bufs=2, space="PSUM") as pqe,
            tc.tile_pool(name="pqs", bufs=1, space="PSUM") as pqs,
            tc.tile_pool(name="pagg", bufs=2, space="PSUM") as pagg,
        ):
            ident = cp.tile([128, 128], BF16)
            make_identity(nc, ident[:])
            ones1 = cp.tile([1, 128], BF16)
            nc.gpsimd.memset(ones1[:], 1.0)
            magic = cp.tile([128, 1], I32)
            nc.gpsimd.memset(magic[:], 0x5F3759DF)

            for rel in rels:
                # --- per-relation static data
                wqskip = cp.tile([128, 2, 2 * D], BF16, tag="wqskip")
                wkv = cp.tile([128, 2, 2 * D], BF16, tag="wkv")
                wmsg = cp.tile([128, 2, D], BF16, tag="wmsg")
                for c in range(2):
                    nc.sync.dma_start(out=wqskip[:, c, :],
                                      in_=rel["wqskip"][c * 128:(c + 1) * 128, :])
                    nc.sync.dma_start(out=wkv[:, c, :],
                                      in_=rel["wkv"][c * 128:(c + 1) * 128, :])
                    nc.sync.dma_start(out=wmsg[:, c, :],
                                      in_=rel["wmsg"][c * 128:(c + 1) * 128, :])
                bskip = cp.tile([1, D], BF16, tag="bskip")
                nc.sync.dma_start(out=bskip[:], in_=rel["bskip"][:])
                if not ln_trivial:
                    gln = cp.tile([128, D], F32, tag="gln")
                    bln = cp.tile([128, D], F32, tag="bln")
                    nc.sync.dma_start(out=gln[:], in_=rel["gln"][:])
                    nc.sync.dma_start(out=bln[:], in_=rel["bln"][:])
                idx16 = cp.tile([128, G, TL // 16], I16, tag="idx16")
                nc.sync.dma_start(out=idx16[:], in_=rel["idx16"][:])
                xdTall = cp.tile([128, G, 2, 128], BF16, tag="xdTall")
                for q in range(0, G, 25):
                    qe_ = min(q + 25, G)
                    nc.sync.dma_start(out=xdTall[:, q:qe_, :, :],
                                      in_=rel["xdT"][:, q:qe_, :, :])

                outd = rel["out"]

                # prefetched loads, issued 2 groups ahead
                PF = 2
                ohs, xgs = {}, {}

                def issue_loads(gg):
                    oh = s3.tile([128, 2, T, 128], BF16, tag="oh")
                    nc.sync.dma_start(
                        out=oh[:].rearrange("p a t j -> p (a t j)"),
                        in_=rel["onehot"][:, gg, :])
                    xgT = s3.tile([128, 2, TL], BF16, tag="xgT")
                    nc.gpsimd.dma_gather(
                        out_ap=xgT[:], in_ap=rel["tabs"][gg // SEGG],
                        idxs_ap=idx16[:, gg, :],
                        num_idxs=TL, num_idxs_reg=TL,
                        elem_size=D, transpose=True)
                    ohs[gg], xgs[gg] = oh, xgT

                for gg in range(min(PF, G)):
                    issue_loads(gg)

                for g in range(G):
                    if g + PF < G:
                        issue_loads(g + PF)
                    oh, xgT = ohs.pop(g), xgs.pop(g)

                    # ---- Q projection for this group's dsts
                    q_ps = pqe.tile([128, D], F32, tag="qe")
                    for c in range(2):
                        nc.tensor.matmul(out=q_ps[:],
                                         lhsT=xdTall[:, g, c, :],
                                         rhs=wqskip[:, c, :D],
                                         start=(c == 0), stop=(c == 1))
                    qg = sp.tile([128, D], BF16, tag="qg")
                    nc.scalar.copy(qg[:], q_ps[:])

                    agg_ps = pagg.tile([128, D + H], F32, tag="agg")

                    for t in range(T):
                        kv_ps = pkv.tile([128, 2 * D], F32, tag="kv")
                        for c in range(2):
                            nc.tensor.matmul(
                                out=kv_ps[:],
                                lhsT=xgT[:, c, ds(t * 128, 128)],
                                rhs=wkv[:, c, :],
                                start=(c == 0), stop=(c == 1))
                        qe_ps = pqe.tile([128, D], F32, tag="qe")
                        nc.tensor.matmul(out=qe_ps[:], lhsT=oh[:, 1, t, :],
                                         rhs=qg[:], start=True, stop=True)

                        kv_sb = sp.tile([128, 2 * D], BF16, tag="kv_sb")
                        nc.vector.tensor_copy(kv_sb[:], kv_ps[:])
                        qkm = sp.tile([128, D], F32, tag="qkm")
                        nc.vector.tensor_tensor(out=qkm[:], in0=qe_ps[:],
                                                in1=kv_sb[:, :D], op=OP.mult)
                        attn = sp.tile([128, H], F32, tag="attn")
                        nc.vector.tensor_reduce(
                            out=attn[:],
                            in_=qkm[:].rearrange("p (h j) -> p h j", j=DH),
                            axis=mybir.AxisListType.X, op=OP.add)
                        wVae = sp.tile([128, D + H], BF16, tag="wVae")
                        nc.scalar.activation(wVae[:, D:], attn[:], AF.Exp)
                        nc.vector.tensor_tensor(
                            out=wVae[:, :D].rearrange("p (h j) -> p h j", j=DH),
                            in0=wVae[:, D:, None].to_broadcast([128, H, DH]),
                            in1=kv_sb[:, D:].rearrange("p (h j) -> p h j", j=DH),
                            op=OP.mult)

                        nc.tensor.matmul(out=agg_ps[:], lhsT=oh[:, 0, t, :],
                                         rhs=wVae[:],
                                         start=(t == 0), stop=(t == T - 1))

                    # ---- normalize + transpose agg
                    rec = sp.tile([128, H], F32, tag="rec")
                    nc.vector.tensor_scalar(out=rec[:], in0=agg_ps[:, D:],
                                            scalar1=1e-30, scalar2=None,
                                            op0=OP.add)
                    nc.vector.reciprocal(rec[:], rec[:])
                    aggn = sp.tile([128, D], BF16, tag="aggn")
                    nc.vector.tensor_tensor(
                        out=aggn[:].rearrange("p (h j) -> p h j", j=DH),
                        in0=agg_ps[:, :D].rearrange("p (h j) -> p h j", j=DH),
                        in1=rec[:, :, None].to_broadcast([128, H, DH]),
                        op=OP.mult)
                    aggT_ps = ptp.tile([128, 2, 128], BF16, tag="tp")
                    for c in range(2):
                        nc.tensor.transpose(out=aggT_ps[:, c, :],
                                            in_=aggn[:, c * 128:(c + 1) * 128],
                                            identity=ident[:])
                    aggT = sp.tile([128, 2, 128], BF16, tag="aggT")
                    nc.vector.tensor_copy(aggT[:], aggT_ps[:])

                    # ---- y = skip + bias + agg@Wmsg ; relu; layernorm
                    skip_ps = pqs.tile([128, D], F32, tag="qskip")
                    for c in range(2):
                        nc.tensor.matmul(out=skip_ps[:],
                                         lhsT=xdTall[:, g, c, :],
                                         rhs=wqskip[:, c, D:],
                                         start=(c == 0), stop=False)
                    nc.tensor.matmul(out=skip_ps[:], lhsT=ones1[:],
                                     rhs=bskip[:], start=False, stop=False)
                    for c in range(2):
                        nc.tensor.matmul(out=skip_ps[:], lhsT=aggT[:, c, :],
                                         rhs=wmsg[:, c, :], start=False,
                                         stop=(c == 1))

                    zr = sp.tile([128, D], F32, tag="zr")
                    msum = sp.tile([128, 1], F32, tag="msum")
                    nc.scalar.activation(zr[:], skip_ps[:], AF.Relu,
                                         accum_out=msum[:, :1])
                    mb = sp.tile([128, 1], F32, tag="mb")
                    nc.vector.tensor_scalar(out=mb[:], in0=msum[:],
                                            scalar1=-1.0 / D, scalar2=None,
                                            op0=OP.mult)
                    sq = sp.tile([128, D], F32, tag="sq")
                    vs = sp.tile([128, 1], F32, tag="vs")
                    nc.scalar.activation(sq[:], zr[:], AF.Square,
                                         accum_out=vs[:, :1])
                    # var = E[z^2] - m^2 (+eps)
                    varp = sp.tile([128, 1], F32, tag="varp")
                    nc.vector.tensor_scalar(out=varp[:], in0=vs[:],
                                            scalar1=1.0 / D, scalar2=EPS,
                                            op0=OP.mult, op1=OP.add)
                    m2 = sp.tile([128, 1], F32, tag="m2")
                    nc.vector.tensor_tensor(out=m2[:], in0=mb[:], in1=mb[:],
                                            op=OP.mult)
                    nc.vector.tensor_tensor(out=varp[:], in0=varp[:],
                                            in1=m2[:], op=OP.subtract)
                    # rstd = 1/sqrt(varp): magic + 2 Newton steps (DVE only)
                    ji = sp.tile([128, 1], I32, tag="ji")
                    nc.vector.tensor_scalar(out=ji[:], in0=varp[:].bitcast(I32),
                                            scalar1=1, scalar2=None,
                                            op0=OP.logical_shift_right)
                    rstd = sp.tile([128, 1], F32, tag="rstd")
                    nc.vector.tensor_tensor(out=rstd[:].bitcast(I32),
                                            in0=magic[:], in1=ji[:],
                                            op=OP.subtract)
                    nt = sp.tile([128, 1], F32, tag="nt")
                    for _ in range(2):
                        nc.vector.tensor_tensor(out=nt[:], in0=rstd[:],
                                                in1=rstd[:], op=OP.mult)
                        nc.vector.tensor_tensor(out=nt[:], in0=nt[:],
                                                in1=varp[:], op=OP.mult)
                        nc.vector.tensor_scalar(out=nt[:], in0=nt[:],
                                                scalar1=-0.5, scalar2=1.5,
                                                op0=OP.mult, op1=OP.add)
                        nc.vector.tensor_tensor(out=rstd[:], in0=rstd[:],
                                                in1=nt[:], op=OP.mult)
                    bias2 = sp.tile([128, 1], F32, tag="bias2")
                    nc.vector.tensor_tensor(out=bias2[:], in0=mb[:],
                                            in1=rstd[:], op=OP.mult)
                    fin = sp.tile([128, D], F32, tag="fin")
                    nc.scalar.activation(fin[:], zr[:], AF.Identity,
                                         bias=bias2[:, :1], scale=rstd[:, :1])
                    if not ln_trivial:
                        fin2 = sp.tile([128, D], F32, tag="fin2")
                        nc.vector.tensor_tensor(out=fin2[:], in0=fin[:],
                                                in1=gln[:], op=OP.mult)
                        nc.vector.tensor_tensor(out=fin2[:], in0=fin2[:],
                                                in1=bln[:], op=OP.add)
                        fin = fin2
                    nc.sync.dma_start(out=outd[ds(g * 128, 128), :], in_=fin[:])
    nc.compile()
    return nc


# ------------------------------------------------------------------- driver

def _sigmoid(x):
    return 1.0 / (1.0 + np.exp(-x))


TRACE = False
LAST = None


def kernel(x_a, x_b, Wq_a, Wk_a, Wv_a, Wq_b, Wk_b, Wv_b,
           Wskip_a_w, Wskip_a_b, Wskip_b_w, Wskip_b_b,
           g_a, b_a, g_b, b_b, mu_ab, Wmsg_ab, mu_ba, Wmsg_ba,
           ei_ab, ei_ba):
    from concourse.bass_utils import run_bass_kernel_spmd

    x_a = np.asarray(x_a, np.float32)
    x_b = np.asarray(x_b, np.float32)
    SCALE = DH ** -0.5

    cap = max(_edge_capacity(np.asarray(ei_ab[1])),
              _edge_capacity(np.asarray(ei_ba[1])))
    T = max(1, -(-cap // 128))

    src_ab, dloc_ab = _pack_edges(np.asarray(ei_ab[0]), np.asarray(ei_ab[1]), T)
    src_ba, dloc_ba = _pack_edges(np.asarray(ei_ba[0]), np.asarray(ei_ba[1]), T)

    xa_bf = x_a.astype(BF)
    xb_bf = x_b.astype(BF)
    tabs_ab, idx_ab = _host_tables(src_ab, xa_bf, T)   # ab: src type a
    tabs_ba, idx_ba = _host_tables(src_ba, xb_bf, T)
    oh_ab = _host_onehots(dloc_ab, T)
    oh_ba = _host_onehots(dloc_ba, T)
    xdT_a = _host_xdT(x_a)
    xdT_b = _host_xdT(x_b)

    def fold_q(Wq, mu):
        s = (SCALE * _sigmoid(np.asarray(mu, np.float64))).astype(np.float32)
        return np.asarray(Wq, np.float32) * np.repeat(s, DH)[None, :]

    def cat_bf(a, b):
        return np.ascontiguousarray(
            np.concatenate([np.asarray(a, np.float32),
                            np.asarray(b, np.float32)], axis=1).astype(BF))

    bc = lambda v: np.broadcast_to(np.asarray(v, np.float32)[None, :], (128, D)).copy()

    ln_trivial = bool(
        np.all(np.asarray(g_a) == 1.0) and np.all(np.asarray(b_a) == 0.0)
        and np.all(np.asarray(g_b) == 1.0) and np.all(np.asarray(b_b) == 0.0))

    shared = {
        # relation ab: src a -> dst b (out_b)
        "wqskip_ab": cat_bf(fold_q(Wq_b, mu_ab), Wskip_b_w),
        "wkv_ab": cat_bf(Wk_a, Wv_a),
        "wmsg_ab": np.asarray(Wmsg_ab, np.float32).astype(BF),
        "bskip_ab": np.asarray(Wskip_b_b, np.float32).astype(BF).reshape(1, D),
        # relation ba: src b -> dst a (out_a)
        "wqskip_ba": cat_bf(fold_q(Wq_a, mu_ba), Wskip_a_w),
        "wkv_ba": cat_bf(Wk_b, Wv_b),
        "wmsg_ba": np.asarray(Wmsg_ba, np.float32).astype(BF),
        "bskip_ba": np.asarray(Wskip_a_b, np.float32).astype(BF).reshape(1, D),
    }
    if not ln_trivial:
        shared.update({
            "gln_ab": bc(g_b), "bln_ab": bc(b_b),
            "gln_ba": bc(g_a), "bln_ba": bc(b_a),
        })
    in_maps = []
    for m in range(M):
        im = dict(shared)
        im["tabs_ab"] = tabs_ab[m]
        im["idx16_ab"] = idx_ab[m]
        im["onehot_ab"] = oh_ab[m]
        im["xdT_ab"] = xdT_b[m]       # dst of ab is type b
        im["tabs_ba"] = tabs_ba[m]
        im["idx16_ba"] = idx_ba[m]
        im["onehot_ba"] = oh_ba[m]
        im["xdT_ba"] = xdT_a[m]
        in_maps.append(im)

    nc = build_program(T, ln_trivial)
    res = run_bass_kernel_spmd(nc, in_maps, list(range(M)), trace=TRACE)
    global LAST
    LAST = res
    out_a = np.empty((N, D), np.float32)
    out_b = np.empty((N, D), np.float32)
    for m in range(M):
        out_b[m * NSH:(m + 1) * NSH] = res.results[m]["out_ab"][:NSH]
        out_a[m * NSH:(m + 1) * NSH] = res.results[m]["out_ba"][:NSH]
    return out_a, out_b


# revision 15
# speedup vs baseline: 4.7497x; 1.0729x over previous
"""HGT layer (2 node types, 2 relations) on 8 Trainium2 cores.

Strategy (dst-sharded, single fused pass, bf16 datapath):
  - Each core owns 12500 destination nodes of each type. Edges are
    partitioned by destination shard on the host and sorted into groups
    of 128 consecutive destination nodes, padded to a fixed per-group
    edge capacity C = T*128 (slot s -> partition s%128, column s//128).
  - Groups are banded into 4 segments of ~25 groups; for each segment the
    host builds a deduplicated source-row table (bf16) plus int16 local
    indices, so the per-group source gather is ONE transposed dma_gather
    (InstDMAGatherAnt) that lands K-contraction-major tiles directly --
    no on-chip transposes of gathered rows.
  - The host also supplies, per group: pre-transposed dst rows (for the
    Q/skip matmul lhsT) and both one-hot orientations (edge x dst-local).
  - Per group: [K|V] projection with one 512-wide bf16 matmul per
    128-chunk; Q projection; Q expanded to edges via the one-hot matmul;
    per-edge logits (DVE mult reading both PSUM operands + per-head
    reduce on GpSimd), exp (ACT); numerator and softmax denominator in
    one accumulating matmul per edge column:
       agg[d, 0:256 | 256:264] += oT[t].T @ [a*V | a].
    Normalization, Wmsg mixing, skip+bias, relu and layernorm fused in
    the same iteration.  LayerNorm uses var = E[z^2] - m^2 and a
    magic-constant Newton rsqrt on DVE, so the Scalar engine needs only
    one activation-table set (exp) for the whole kernel.
  - Fully static unroll (no For_i back-edge barriers); PSUM pools sized
    so consecutive groups overlap (kv/qe/agg double-buffered).
"""

import numpy as np
import ml_dtypes

import concourse.bacc as bacc
import concourse.bass as bass
import concourse.mybir as mybir
import concourse.tile as tile
from concourse.bass import ds
from concourse.masks import make_identity

N = 100000
D = 256
H = 8
DH = 32
M = 8            # cores
NSH = N // M     # 12500 dst rows per core per type
G = 98           # dst groups of 128 per core (98*128 = 12544)
NPAD = G * 128   # 12544
NSEG = 4         # source-table segments (groups g//25)
SEGG = 25        # groups per segment
U = 12288        # table rows per segment (>= max unique sources + pad)
EPS = 1e-5
F32 = mybir.dt.float32
BF16 = mybir.dt.bfloat16
I32 = mybir.dt.int32
I16 = mybir.dt.int16
AF = mybir.ActivationFunctionType
OP = mybir.AluOpType
BF = ml_dtypes.bfloat16


# ----------------------------------------------------------------- host prep

def _pack_edges(src, dst, T):
    """Partition edges by dst shard into groups of 128 dsts with T*128 slots.
    Returns src_idx [M, G, T, 128] int64 and dloc [M, G, T, 128] int64
    (slot (t, e): partition e, column t; dloc 999 for padding, src 0)."""
    order = np.argsort(dst, kind="stable")
    s_sorted = src[order].astype(np.int64)
    d_sorted = dst[order].astype(np.int64)

    core = d_sorted // NSH
    local = d_sorted - core * NSH
    grp = local // 128
    dloc = local - grp * 128
    key = core * G + grp
    first = np.r_[0, np.flatnonzero(np.diff(key)) + 1]
    starts = np.zeros(len(key), dtype=np.int64)
    starts[first] = first
    starts = np.maximum.accumulate(starts)
    slot = np.arange(len(key), dtype=np.int64) - starts

    maxslot = int(slot.max()) if len(slot) else 0
    assert maxslot < T * 128, f"edge capacity exceeded: {maxslot + 1} > {T * 128}"

    src_arr = np.zeros((M * G, T * 128), dtype=np.int64)
    dst_arr = np.full((M * G, T * 128), 999, dtype=np.int64)
    # slot s -> (t = s // 128, e = s % 128): flat index t*128+e = s
    src_arr[key, slot] = s_sorted
    dst_arr[key, slot] = dloc
    return (src_arr.reshape(M, G, T, 128), dst_arr.reshape(M, G, T, 128))


def _edge_capacity(dst):
    d = np.sort(dst.astype(np.int64))
    core = d // NSH
    grp = (d - core * NSH) // 128
    key = core * G + grp
    _, counts = np.unique(key, return_counts=True)
    return int(counts.max())


def _host_tables(src_idx, x_bf, T):
    """Per (core, segment) deduplicated source tables + int16 indices.
    Returns tabs [M, NSEG, U, D] bf16 and idx16 [M, 128, G, T*128//16] i16."""
    tabs = np.zeros((M, NSEG, U, D), dtype=BF)
    idxw = np.zeros((M, 128, G, T * 128 // 16), dtype=np.int16)
    for m in range(M):
        for s in range(NSEG):
            g0, g1 = s * SEGG, min((s + 1) * SEGG, G)
            srcs = src_idx[m, g0:g1].reshape(-1)          # [(g1-g0)*T*128]
            uniq, inv = np.unique(srcs, return_inverse=True)
            assert len(uniq) <= U, f"segment table overflow: {len(uniq)} > {U}"
            tabs[m, s, :len(uniq)] = x_bf[uniq]
            inv = inv.astype(np.int16).reshape(g1 - g0, T * 128)
            # index i lives at [i % 16, i // 16]
            iw = inv.reshape(g1 - g0, T * 128 // 16, 16)
            idxw[m, :16, g0:g1] = iw.transpose(2, 0, 1)
    idxw[:, 16:] = np.tile(idxw[:, :16], (1, 7, 1, 1))
    return tabs, idxw


def _host_onehots(dloc, T):
    """[M, G, T, 128] dloc -> onehot [M, 128, G, 2*T*128] bf16:
    [:, :, g, 0:T*128] = oT (partition = edge slot), [T*128:] = od."""
    oh = np.zeros((M, 128, G, 2 * T * 128), dtype=BF)
    rng = np.arange(128)
    for m in range(M):
        # oT[e, t*128+j] = (dloc[g, t, e] == j)
        oT = (dloc[m][:, :, :, None] == rng[None, None, None, :])  # [G,T,128e,128j]
        oh[m, :, :, :T * 128] = oT.transpose(2, 0, 1, 3).reshape(128, G, T * 128)
        od = oT.transpose(3, 0, 1, 2)   # [j, G, T, e]
        oh[m, :, :, T * 128:] = od.reshape(128, G, T * 128)
    return oh


def _host_xdT(x):
    """[N, D] f32 -> [M, 128, G, 2, 128] bf16 transposed dst rows:
    [:, dmod, g, c, j] = x[m*NSH + g*128 + j, c*128 + dmod]."""
    out = np.zeros((M, NPAD, D), dtype=BF)
    for m in range(M):
        out[m, :NSH] = x[m * NSH:(m + 1) * NSH].astype(BF)
    # [M, G, 128j, 2, 128dmod] -> [M, 128dmod, G, 2, 128j]
    v = out.reshape(M, G, 128, 2, 128).transpose(0, 4, 1, 3, 2)
    return np.ascontiguousarray(v)


# ------------------------------------------------------------- bass program

DEBUG = False


def build_program(T, ln_trivial):
    nc = bacc.Bacc("TRN2", target_bir_lowering=False, debug=False)

    def drt(name, shape, dtype=F32, kind="ExternalInput"):
        return nc.dram_tensor(name, shape, dtype, kind=kind)

    TL = T * 128

    rels = []
    for r in ("ab", "ba"):
        rels.append(dict(
            name=r,
            tabs=drt(f"tabs_{r}", [NSEG, U, D], BF16),
            idx16=drt(f"idx16_{r}", [128, G, TL // 16], I16),
            onehot=drt(f"onehot_{r}", [128, G, 2 * TL], BF16),
            xdT=drt(f"xdT_{r}", [128, G, 2, 128], BF16),
            wqskip=drt(f"wqskip_{r}", [D, 2 * D], BF16),  # [Wq | Wskip]
            wkv=drt(f"wkv_{r}", [D, 2 * D], BF16),        # [Wk | Wv]
            wmsg=drt(f"wmsg_{r}", [D, D], BF16),
            bskip=drt(f"bskip_{r}", [1, D], BF16),
            out=drt(f"out_{r}", [NPAD, D], kind="ExternalOutput"),
        ))
        if not ln_trivial:
            rels[-1]["gln"] = drt(f"gln_{r}", [128, D])
            rels[-1]["bln"] = drt(f"bln_{r}", [128, D])

    with tile.TileContext(nc) as tc:
        with (
            tc.tile_pool(name="const", bufs=1) as cp,
            tc.tile_pool(name="sbuf", bufs=2) as sp,
            tc.tile_pool(name="sb3", bufs=3) as s3,
            tc.tile_pool(name="ptp", bufs=1, space="PSUM") as ptp,
            tc.tile_pool(name="pkv", bufs=2, space="PSUM") as pkv,
            tc.tile_pool(name="pqe", bufs=2, space="PSUM") as pqe,
            tc.tile_pool(name="pqs", bufs=1, space="PSUM") as pqs,
            tc.tile_pool(name="pagg", bufs=2, space="PSUM") as pagg,
        ):
            ident = cp.tile([128, 128], BF16)
            make_identity(nc, ident[:])
            ones1 = cp.tile([1, 128], BF16)
            nc.gpsimd.memset(ones1[:], 1.0)
            magic = cp.tile([128, 1], I32)
            nc.gpsimd.memset(magic[:], 0x5F3759DF)

            for rel in rels:
                # --- per-relation static data
                wqskip = cp.tile([128, 2, 2 * D], BF16, tag="wqskip")
                wkv = cp.tile([128, 2, 2 * D], BF16, tag="wkv")
                wmsg = cp.tile([128, 2, D], BF16, tag="wmsg")
                for c in range(2):
                    nc.sync.dma_start(out=wqskip[:, c, :],
                                      in_=rel["wqskip"][c * 128:(c + 1) * 128, :])
                    nc.sync.dma_start(out=wkv[:, c, :],
                                      in_=rel["wkv"][c * 128:(c + 1) * 128, :])
                    nc.sync.dma_start(out=wmsg[:, c, :],
                                      in_=rel["wmsg"][c * 128:(c + 1) * 128, :])
                bskip = cp.tile([1, D], BF16, tag="bskip")
                nc.sync.dma_start(out=bskip[:], in_=rel["bskip"][:])
                if not ln_trivial:
                    gln = cp.tile([128, D], F32, tag="gln")
                    bln = cp.tile([128, D], F32, tag="bln")
                    nc.sync.dma_start(out=gln[:], in_=rel["gln"][:])
                    nc.sync.dma_start(out=bln[:], in_=rel["bln"][:])
                idx16 = cp.tile([128, G, TL // 16], I16, tag="idx16")
                nc.sync.dma_start(out=idx16[:], in_=rel["idx16"][:])
                xdTall = cp.tile([128, G, 2, 128], BF16, tag="xdTall")
                for q in range(0, G, 25):
                    qe_ = min(q + 25, G)
                    nc.sync.dma_start(out=xdTall[:, q:qe_, :, :],
                                      in_=rel["xdT"][:, q:qe_, :, :])

                outd = rel["out"]

                # prefetched loads, issued 2 groups ahead
                PF = 2
                ohs, xgs = {}, {}

                def issue_loads(gg):
                    oh = s3.tile([128, 2, T, 128], BF16, tag="oh")
                    nc.sync.dma_start(
                        out=oh[:].rearrange("p a t j -> p (a t j)"),
                        in_=rel["onehot"][:, gg, :])
                    xgT = s3.tile([128, 2, TL], BF16, tag="xgT")
                    nc.gpsimd.dma_gather(
                        out_ap=xgT[:], in_ap=rel["tabs"][gg // SEGG],
                        idxs_ap=idx16[:, gg, :],
                        num_idxs=TL, num_idxs_reg=TL,
                        elem_size=D, transpose=True)
                    ohs[gg], xgs[gg] = oh, xgT

                for gg in range(min(PF, G)):
                    issue_loads(gg)

                for g in range(G):
                    if g + PF < G:
                        issue_loads(g + PF)
                    oh, xgT = ohs.pop(g), xgs.pop(g)

                    # ---- Q projection for this group's dsts
                    q_ps = pqe.tile([128, D], F32, tag="qe")
                    for c in range(2):
                        nc.tensor.matmul(out=q_ps[:],
                                         lhsT=xdTall[:, g, c, :],
                                         rhs=wqskip[:, c, :D],
                                         start=(c == 0), stop=(c == 1))
                    qg = sp.tile([128, D], BF16, tag="qg")
                    nc.scalar.copy(qg[:], q_ps[:])

                    agg_ps = pagg.tile([128, D + H], F32, tag="agg")

                    for t in range(T):
                        kv_ps = pkv.tile([128, 2 * D], F32, tag="kv")
                        for c in range(2):
                            nc.tensor.matmul(
                                out=kv_ps[:],
                                lhsT=xgT[:, c, ds(t * 128, 128)],
                                rhs=wkv[:, c, :],
                                start=(c == 0), stop=(c == 1))
                        qe_ps = pqe.tile([128, D], F32, tag="qe")
                        nc.tensor.matmul(out=qe_ps[:], lhsT=oh[:, 1, t, :],
                                         rhs=qg[:], start=True, stop=True)

                        k_sb = sp.tile([128, D], BF16, tag="k_sb")
                        nc.scalar.copy(k_sb[:], kv_ps[:, :D])
                        qkm = sp.tile([128, D], F32, tag="qkm")
                        nc.vector.tensor_tensor(out=qkm[:], in0=qe_ps[:],
                                                in1=k_sb[:], op=OP.mult)
                        attn = sp.tile([128, H], F32, tag="attn")
                        nc.vector.tensor_reduce(
                            out=attn[:],
                            in_=qkm[:].rearrange("p (h j) -> p h j", j=DH),
                            axis=mybir.AxisListType.X, op=OP.add)
                        wVae = sp.tile([128, D + H], BF16, tag="wVae")
                        nc.scalar.activation(wVae[:, D:], attn[:], AF.Exp)
                        nc.vector.tensor_tensor(
                            out=wVae[:, :D].rearrange("p (h j) -> p h j", j=DH),
                            in0=wVae[:, D:, None].to_broadcast([128, H, DH]),
                            in1=kv_ps[:, D:].rearrange("p (h j) -> p h j", j=DH),
                            op=OP.mult)

                        nc.tensor.matmul(out=agg_ps[:], lhsT=oh[:, 0, t, :],
                                         rhs=wVae[:],
                                         start=(t == 0), stop=(t == T - 1))

                    # ---- normalize + transpose agg
                    rec = sp.tile([128, H], F32, tag="rec")
                    nc.vector.tensor_scalar(out=rec[:], in0=agg_ps[:, D:],
                                            scalar1=1e-30, scalar2=None,
                                            op0=OP.add)
                    nc.vector.reciprocal(rec[:], rec[:])
                    aggn = sp.tile([128, D], BF16, tag="aggn")
                    nc.vector.tensor_tensor(
                        out=aggn[:].rearrange("p (h j) -> p h j", j=DH),
                        in0=agg_ps[:, :D].rearrange("p (h j) -> p h j", j=DH),
                        in1=rec[:, :, None].to_broadcast([128, H, DH]),
                        op=OP.mult)
                    aggT_ps = ptp.tile([128, 2, 128], BF16, tag="tp")
                    for c in range(2):
                        nc.tensor.transpose(out=aggT_ps[:, c, :],
                                            in_=aggn[:, c * 128:(c + 1) * 128],
                                            identity=ident[:])
                    aggT = sp.tile([128, 2, 128], BF16, tag="aggT")
                    nc.vector.tensor_copy(aggT[:], aggT_ps[:])

                    # ---- y = skip + bias + agg@Wmsg ; relu; layernorm
                    skip_ps = pqs.tile([128, D], F32, tag="qskip")
                    for c in range(2):
                        nc.tensor.matmul(out=skip_ps[:],
                                         lhsT=xdTall[:, g, c, :],
                                         rhs=wqskip[:, c, D:],
                                         start=(c == 0), stop=False)
                    nc.tensor.matmul(out=skip_ps[:], lhsT=ones1[:],
                                     rhs=bskip[:], start=False, stop=False)
                    for c in range(2):
                        nc.tensor.matmul(out=skip_ps[:], lhsT=aggT[:, c, :],
                                         rhs=wmsg[:, c, :], start=False,
                                         stop=(c == 1))

                    zr = sp.tile([128, D], F32, tag="zr")
                    msum = sp.tile([128, 1], F32, tag="msum")
                    nc.scalar.activation(zr[:], skip_ps[:], AF.Relu,
                                         accum_out=msum[:, :1])
                    mb = sp.tile([128, 1], F32, tag="mb")
                    nc.vector.tensor_scalar(out=mb[:], in0=msum[:],
                                            scalar1=-1.0 / D, scalar2=None,
                                            op0=OP.mult)
                    sq = sp.tile([128, D], F32, tag="sq")
                    vs = sp.tile([128, 1], F32, tag="vs")
                    nc.scalar.activation(sq[:], zr[:], AF.Square,
                                         accum_out=vs[:, :1])
                    # var = E[z^2] - m^2 (+eps)
                    varp = sp.tile([128, 1], F32, tag="varp")
                    nc.vector.tensor_scalar(out=varp[:], in0=vs[:],
                                            scalar1=1.0 / D, scalar2=EPS,
                                            op0=OP.mult, op1=OP.add)
                    m2 = sp.tile([128, 1], F32, tag="m2")
                    nc.vector.tensor_tensor(out=m2[:], in0=mb[:], in1=mb[:],
                                            op=OP.mult)
                    nc.vector.tensor_tensor(out=varp[:], in0=varp[:],
                                            in1=m2[:], op=OP.subtract)
                    # rstd = 1/sqrt(varp): magic + 2 Newton steps (DVE only)
                    ji = sp.tile([128, 1], I32, tag="ji")
                    nc.vector.tensor_scalar(out=ji[:], in0=varp[:].bitcast(I32),
                                            scalar1=1, scalar2=None,
                                            op0=OP.logical_shift_right)
                    rstd = sp.tile([128, 1], F32, tag="rstd")
                    nc.vector.tensor_tensor(out=rstd[:].bitcast(I32),
                                            in0=magic[:], in1=ji[:],
                                            op=OP.subtract)
                    nt = sp.tile([128, 1], F32, tag="nt")
                    for _ in range(1):
                        nc.vector.tensor_tensor(out=nt[:], in0=rstd[:],
                                                in1=rstd[:], op=OP.mult)
                        nc.vector.tensor_tensor(out=nt[:], in0=nt[:],
                                                in1=varp[:], op=OP.mult)
                        nc.vector.tensor_scalar(out=nt[:], in0=nt[:],
                                                scalar1=-0.5, scalar2=1.5,
                                                op0=OP.mult, op1=OP.add)
                        nc.vector.tensor_tensor(out=rstd[:], in0=rstd[:],
                                                in1=nt[:], op=OP.mult)
                    bias2 = sp.tile([128, 1], F32, tag="bias2")
                    nc.vector.tensor_tensor(out=bias2[:], in0=mb[:],
                                            in1=rstd[:], op=OP.mult)
                    fin = sp.tile([128, D], F32, tag="fin")
                    nc.scalar.activation(fin[:], zr[:], AF.Identity,
                                         bias=bias2[:, :1], scale=rstd[:, :1])
                    if not ln_trivial:
                        fin2 = sp.tile([128, D], F32, tag="fin2")
                        nc.vector.tensor_tensor(out=fin2[:], in0=fin[:],
                                                in1=gln[:], op=OP.mult)
                        nc.vector.tensor_tensor(out=fin2[:], in0=fin2[:],
                                                in1=bln[:], op=OP.add)
                        fin = fin2
                    nc.sync.dma_start(out=outd[ds(g * 128, 128), :], in_=fin[:])
    nc.compile()
    return nc


# ------------------------------------------------------------------- driver

def _sigmoid(x):
    return 1.0 / (1.0 + np.exp(-x))


TRACE = False
LAST = None


def kernel(x_a, x_b, Wq_a, Wk_a, Wv_a, Wq_b, Wk_b, Wv_b,
           Wskip_a_w, Wskip_a_b, Wskip_b_w, Wskip_b_b,
           g_a, b_a, g_b, b_b, mu_ab, Wmsg_ab, mu_ba, Wmsg_ba,
           ei_ab, ei_ba):
    from concourse.bass_utils import run_bass_kernel_spmd

    x_a = np.asarray(x_a, np.float32)
    x_b = np.asarray(x_b, np.float32)
    SCALE = DH ** -0.5

    cap = max(_edge_capacity(np.asarray(ei_ab[1])),
              _edge_capacity(np.asarray(ei_ba[1])))
    T = max(1, -(-cap // 128))

    src_ab, dloc_ab = _pack_edges(np.asarray(ei_ab[0]), np.asarray(ei_ab[1]), T)
    src_ba, dloc_ba = _pack_edges(np.asarray(ei_ba[0]), np.asarray(ei_ba[1]), T)

    xa_bf = x_a.astype(BF)
    xb_bf = x_b.astype(BF)
    tabs_ab, idx_ab = _host_tables(src_ab, xa_bf, T)   # ab: src type a
    tabs_ba, idx_ba = _host_tables(src_ba, xb_bf, T)
    oh_ab = _host_onehots(dloc_ab, T)
    oh_ba = _host_onehots(dloc_ba, T)
    xdT_a = _host_xdT(x_a)
    xdT_b = _host_xdT(x_b)

    def fold_q(Wq, mu):
        s = (SCALE * _sigmoid(np.asarray(mu, np.float64))).astype(np.float32)
        return np.asarray(Wq, np.float32) * np.repeat(s, DH)[None, :]

    def cat_bf(a, b):
        return np.ascontiguousarray(
            np.concatenate([np.asarray(a, np.float32),
                            np.asarray(b, np.float32)], axis=1).astype(BF))

    bc = lambda v: np.broadcast_to(np.asarray(v, np.float32)[None, :], (128, D)).copy()

    ln_trivial = bool(
        np.all(np.asarray(g_a) == 1.0) and np.all(np.asarray(b_a) == 0.0)
        and np.all(np.asarray(g_b) == 1.0) and np.all(np.asarray(b_b) == 0.0))

    shared = {
        # relation ab: src a -> dst b (out_b)
        "wqskip_ab": cat_bf(fold_q(Wq_b, mu_ab), Wskip_b_w),
        "wkv_ab": cat_bf(Wk_a, Wv_a),
        "wmsg_ab": np.asarray(Wmsg_ab, np.float32).astype(BF),
        "bskip_ab": np.asarray(Wskip_b_b, np.float32).astype(BF).reshape(1, D),
        # relation ba: src b -> dst a (out_a)
        "wqskip_ba": cat_bf(fold_q(Wq_a, mu_ba), Wskip_a_w),
        "wkv_ba": cat_bf(Wk_b, Wv_b),
        "wmsg_ba": np.asarray(Wmsg_ba, np.float32).astype(BF),
        "bskip_ba": np.asarray(Wskip_a_b, np.float32).astype(BF).reshape(1, D),
    }
    if not ln_trivial:
        shared.update({
            "gln_ab": bc(g_b), "bln_ab": bc(b_b),
            "gln_ba": bc(g_a), "bln_ba": bc(b_a),
        })
    in_maps = []
    for m in range(M):
        im = dict(shared)
        im["tabs_ab"] = tabs_ab[m]
        im["idx16_ab"] = idx_ab[m]
        im["onehot_ab"] = oh_ab[m]
        im["xdT_ab"] = xdT_b[m]       # dst of ab is type b
        im["tabs_ba"] = tabs_ba[m]
        im["idx16_ba"] = idx_ba[m]
        im["onehot_ba"] = oh_ba[m]
        im["xdT_ba"] = xdT_a[m]
        in_maps.append(im)

    nc = build_program(T, ln_trivial)
    res = run_bass_kernel_spmd(nc, in_maps, list(range(M)), trace=TRACE)
    global LAST
    LAST = res
    out_a = np.empty((N, D), np.float32)
    out_b = np.empty((N, D), np.float32)
    for m in range(M):
        out_b[m * NSH:(m + 1) * NSH] = res.results[m]["out_ab"][:NSH]
        out_a[m * NSH:(m + 1) * NSH] = res.results[m]["out_ba"][:NSH]
    return out_a, out_b
